# revision 4
# baseline (speedup 1.0000x reference)
"""Multi-head attention Trainium2 kernel v2 (nn_MultiHeadAttention_7035156430929).

B=4, S=1024, E=1024, H=16, D=64. Sharding: 8 cores = 4 batches x 2
head-halves (tensor parallel per the hint). Each core computes 8 heads
(a 512-feature slice of Q/K/V and the matching 512 rows of Wo) for all
1024 queries/keys of its batch, producing a PARTIAL output [1024, 1024].
Host sums the two partials per batch and adds bo + Wo@bv (the softmax
rows sum to 1, so the V bias folds into a constant output bias).

vs the v1 baseline: no duplicated K/V projection work (262144 PE rows
per core instead of 327680), all matmul operands bf16 (half the HBM
traffic, full PE rate), exp in [128,1024] double-PSUM-bank tiles (64
ACT instructions instead of 128), the V ones-column filled by a DVE
copy instead of a 2048-descriptor strided DMA, merged [x.T | w] input
DMAs (36 instead of 64 per iteration), pools/tiles allocated outside
the hardware loop so iterations pipeline (no per-iteration drain), and
a software-pipelined emission order: Q/K projection chains staggered to
hide their evictions, V-projection and previous-head PV matmuls woven
into the exp-paced score loops, and the last head's PV overlapped with
the first output-projection chains.

Engine assignment: PE matmuls; ACT = exp + half the O evictions;
DVE = Q/K/V evictions + reciprocal + normalize mult + half the O
evictions; Pool = denominator partition-broadcast + const/output DMAs;
SP = streamed input loads. PSUM: tag "e" 2x[128,1024] + tag "pv"
2x[65..128,1024] = all 8 banks.
"""
import sys

sys.path.insert(0, "/opt/trn_rl_repo")

from contextlib import ExitStack

import numpy as np
import ml_dtypes

import concourse.bacc as bacc
import concourse.tile as tile
from concourse import mybir
from concourse.bass_utils import run_bass_kernel_spmd

B, S, E, H, D = 4, 1024, 1024, 16, 64
P = 128
F = 512            # features (heads*D) per core
FT = F // P        # 4 feature tiles
ET = E // P        # 8 embedding tiles
NKT = S // P       # 8 key-token tiles
HC = 8             # heads per core
DP1 = D + 1        # V columns per head incl. ones column
N_CORES = 8
F32 = mybir.dt.float32
BF16 = mybir.dt.bfloat16
AF = mybir.ActivationFunctionType
BF16_NP = ml_dtypes.bfloat16


def _declare(nc):
    dp = nc.declare_dram_parameter
    t = {}
    # activations concatenated with this core's weight slice: [x.T | w]
    t["xqw"] = dp("xqw", [E, S + F], BF16, isOutput=False)
    t["xkw"] = dp("xkw", [E, S + F], BF16, isOutput=False)
    t["xvw"] = dp("xvw", [E, S + F], BF16, isOutput=False)
    t["woT"] = dp("woT", [F, E], BF16, isOutput=False)   # (feat, e_out)
    t["bqkm"] = dp("bqkm", [P, 2 * FT + NKT], F32, isOutput=False)
    t["ones"] = dp("ones", [P, HC], BF16, isOutput=False)
    t["out"] = dp("out", [S, E], BF16, isOutput=True)    # PARTIAL output
    return t


class _Tiles:
    pass


def _alloc(nc, tc, ctx):
    """Pools + persistent tiles, created once OUTSIDE the hardware loop so
    iterations pipeline (no per-iteration pool-boundary drain)."""
    a = _Tiles()
    const = ctx.enter_context(tc.tile_pool(name="const", bufs=1))
    a.bqkm_t = const.tile([P, 2 * FT + NKT], F32, tag="bqkm", name="bqkm")
    a.on_t = const.tile([P, HC], BF16, tag="on", name="on")

    xq_p = ctx.enter_context(tc.tile_pool(name="xq", bufs=ET))
    xk_p = ctx.enter_context(tc.tile_pool(name="xk", bufs=ET))
    xv_p = ctx.enter_context(tc.tile_pool(name="xv", bufs=ET))
    wo_p = ctx.enter_context(tc.tile_pool(name="wo", bufs=2))
    qt_p = ctx.enter_context(tc.tile_pool(name="qt", bufs=FT))
    kt_p = ctx.enter_context(tc.tile_pool(name="kt", bufs=FT))
    va_p = ctx.enter_context(tc.tile_pool(name="va", bufs=NKT))
    atn_p = ctx.enter_context(tc.tile_pool(name="atn", bufs=FT))
    a.pt_p = ctx.enter_context(tc.tile_pool(name="pt", bufs=12))
    a.rec_p = ctx.enter_context(tc.tile_pool(name="rec", bufs=2))
    a.bc_p = ctx.enter_context(tc.tile_pool(name="bc", bufs=2))
    a.ob_p = ctx.enter_context(tc.tile_pool(name="ob", bufs=2))
    a.psum = ctx.enter_context(tc.tile_pool(name="psum", bufs=2, space="PSUM"))

    a.xqw = [xq_p.tile([P, S + F], BF16, tag="xq", name="xqw")
             for _ in range(ET)]
    a.xkw = [xk_p.tile([P, S + F], BF16, tag="xk", name="xkw")
             for _ in range(ET)]
    a.xvw = [xv_p.tile([P, S + F], BF16, tag="xv", name="xvw")
             for _ in range(ET)]
    a.xq = [tl[:, 0:S] for tl in a.xqw]
    a.xk = [tl[:, 0:S] for tl in a.xkw]
    a.xv = [tl[:, 0:S] for tl in a.xvw]
    a.wq = [tl[:, S:S + F] for tl in a.xqw]
    a.wk = [tl[:, S:S + F] for tl in a.xkw]
    a.wv = [tl[:, S:S + F] for tl in a.xvw]
    a.wo2 = [wo_p.tile([P, 2 * E], BF16, tag="wo", name="wo2")
             for _ in range(2)]
    a.wo = [a.wo2[ft // 2][:, (ft % 2) * E:(ft % 2 + 1) * E]
            for ft in range(FT)]
    a.QT = [qt_p.tile([P, S], BF16, tag="qt", name="qt") for _ in range(FT)]
    a.KT = [kt_p.tile([P, S], BF16, tag="kt", name="kt") for _ in range(FT)]
    a.VA = [va_p.tile([P, HC * DP1], BF16, tag="va", name="va")
            for _ in range(NKT)]
    a.ATN = [atn_p.tile([P, S], BF16, tag="atn", name="atn")
             for _ in range(FT)]
    return a


def _emit_body(nc, tc, t, a):
    xqw_d, xkw_d, xvw_d = t["xqw"], t["xkw"], t["xvw"]
    woT = t["woT"]
    ones, out = t["ones"], t["out"]
    bq_t = a.bqkm_t[:, 0:FT]
    bk_t = a.bqkm_t[:, FT:2 * FT]
    mb_t = a.bqkm_t[:, 2 * FT:2 * FT + NKT]
    on_t = a.on_t
    pt_p, rec_p, bc_p, ob_p, psum = a.pt_p, a.rec_p, a.bc_p, a.ob_p, a.psum
    xq, xk, xv = a.xq, a.xk, a.xv
    wq, wk, wv, wo = a.wq, a.wk, a.wv, a.wo
    QT, KT, VA, ATN = a.QT, a.KT, a.VA, a.ATN

    nc.gpsimd.dma_start(a.bqkm_t[:], t["bqkm"].ap()[:])
    nc.gpsimd.dma_start(on_t[:], ones.ap()[:])

    # Input loads in consumption order on the SP (sync) queue.
    for et in range(ET):
        nc.sync.dma_start(a.xqw[et][:], xqw_d.ap()[et * P:(et + 1) * P, :])
    for et in range(ET):
        nc.sync.dma_start(a.xkw[et][:], xkw_d.ap()[et * P:(et + 1) * P, :])
    for et in range(ET):
        nc.sync.dma_start(a.xvw[et][:], xvw_d.ap()[et * P:(et + 1) * P, :])
    for wi in range(2):
        nc.sync.dma_start(
            a.wo2[wi][:].rearrange("p (t e) -> p t e", t=2),
            woT.ap()[wi * 2 * P:(wi + 1) * 2 * P, :]
            .rearrange("(t p) e -> p t e", p=P))

    # ---- Q/K projections, interleaved so each pair's PSUM eviction (DVE)
    # hides under the other projection's matmuls: Qp0, Kp0, Qp1, Kp1.
    def proj_pair(w, x, dst, bias, fp):
        # ft0's chain runs 4 et-steps ahead of ft1's, so ft0's eviction is
        # fully hidden under ft1's tail and the next pair never stalls.
        ps = [psum.tile([P, S], F32, tag="e", name="pse") for _ in range(2)]

        def step(f2, et):
            ft = fp * 2 + f2
            for qb in range(2):
                qs = slice(qb * F, (qb + 1) * F)
                nc.tensor.matmul(
                    ps[f2][:, qs], w[et][:, ft * P:(ft + 1) * P],
                    x[et][:, qs],
                    start=(et == 0), stop=(et == ET - 1))

        def evict(f2):
            ft = fp * 2 + f2
            nc.vector.tensor_scalar_add(dst[ft][:], ps[f2][:],
                                        bias[:, ft:ft + 1])

        for et in range(4):
            step(0, et)
        for i in range(4):
            step(0, 4 + i)
            step(1, i)
        evict(0)
        for et in range(4, ET):
            step(1, et)
        evict(1)

    proj_pair(wq, xq, QT, bq_t, 0)
    proj_pair(wk, xk, KT, bk_t, 0)
    proj_pair(wq, xq, QT, bq_t, 1)
    proj_pair(wk, xk, KT, bk_t, 1)

    # ---- V projection work generator: woven into the (ACT-paced) score
    # loops of attention streams 0-1 to fill PE idle slots. Each unit is a
    # couple of matmuls or an eviction; VA[tt] complete before PV(x=0).
    def v_units():
        for tt in range(NKT):
            ps = psum.tile([P, F], F32, tag="e", name="psv")
            for et in range(0, ET, 2):
                def mm2(ps=ps, tt=tt, et=et):
                    for e2 in (et, et + 1):
                        nc.tensor.matmul(
                            ps[:], xv[e2][:, tt * P:(tt + 1) * P], wv[e2][:],
                            start=(e2 == 0), stop=(e2 == ET - 1))
                yield mm2
            def evict(ps=ps, tt=tt):
                va3 = VA[tt][:].rearrange("p (h c) -> p h c", c=DP1)
                ps3 = ps[:].rearrange("p (h c) -> p h c", c=D)
                nc.vector.tensor_copy(va3[:, :, 0:D], ps3)
                nc.vector.tensor_copy(
                    va3[:, :, D:DP1],
                    on_t[:].rearrange("p (h c) -> p h c", c=1))
            yield evict

    vgen = v_units()

    # ---- Attention: stream x = head h (j = x//2 f-tile, hp partition).
    # Stream x's score loop (ACT-paced by exp) has the PV matmuls of
    # stream x-1 and V-projection quanta woven into its PE idle slots.
    all_pts = {}

    def normalize(x, pv):
        j, hh2 = divmod(x, 2)
        hp = hh2 * D
        # q-halves: finer subtile deps let O-proj columns start earlier
        for qb in range(2):
            qs = slice(qb * F, (qb + 1) * F)
            rec = rec_p.tile([1, F], F32, tag="rec", name="rec")
            bc = bc_p.tile([D, F], F32, tag="bc", name="bc")
            nc.vector.reciprocal(rec[:], pv[D:DP1, qs])
            nc.gpsimd.partition_broadcast(bc[:], rec[:])
            nc.vector.tensor_mul(ATN[j][hp:hp + D, qs], pv[0:D, qs], bc[:])

    def emit_stream(x):
        j, hh2 = divmod(x, 2)
        hp = hh2 * D
        prev = all_pts.pop(x - 1, None)
        if prev is not None:
            pv_prev = psum.tile([DP1, S], F32, tag="pv", name="pspv", bufs=2)
        pts = []
        for kt in range(NKT):
            pe = psum.tile([P, S], F32, tag="e", name="pse")
            for qb in range(2):
                nc.tensor.matmul(
                    pe[:, qb * F:(qb + 1) * F],
                    KT[j][hp:hp + D, kt * P:(kt + 1) * P],
                    QT[j][hp:hp + D, qb * F:(qb + 1) * F],
                    start=True, stop=True, tile_position=(hp, 0))
            pt = pt_p.tile([P, S], BF16, tag="pt", name="pt")
            nc.scalar.activation(pt[:], pe[:], AF.Exp,
                                 bias=mb_t[:, kt:kt + 1])
            pts.append(pt)
            if prev is not None:
                for qb in range(2):
                    qs = slice(qb * F, (qb + 1) * F)
                    nc.tensor.matmul(
                        pv_prev[:, qs],
                        VA[kt][:, (x - 1) * DP1:x * DP1], prev[kt][:, qs],
                        start=(kt == 0), stop=(kt == NKT - 1))
            for _ in range(3):
                u = next(vgen, None)
                if u is not None:
                    u()
        if prev is not None:
            normalize(x - 1, pv_prev)
        all_pts[x] = pts

    for x in range(HC):
        emit_stream(x)
        if x == 1:
            # all V-projection units must precede the first PV reads
            assert next(vgen, None) is None, "V weave not exhausted"
    # final stream's PV runs unwoven (O-proj matmuls queue right behind)
    x = HC - 1
    pts = all_pts.pop(x)
    pv = psum.tile([DP1, S], F32, tag="pv", name="pspv", bufs=2)
    for kt in range(NKT):
        for qb in range(2):
            qs = slice(qb * F, (qb + 1) * F)
            nc.tensor.matmul(
                pv[:, qs], VA[kt][:, x * DP1:(x + 1) * DP1], pts[kt][:, qs],
                start=(kt == 0), stop=(kt == NKT - 1))
    normalize(x, pv)

    # ---- Output projection: out[tt] = sum_ft ATN[ft][:,tt].T @ wo[ft].
    # Alternate e/pv PSUM tags for a deeper eviction rotation. The first
    # two chains' ft0-2 steps are emitted before any ft3 step, filling PE
    # while the final stream's normalize chain completes ATN[3].
    def o_mm(po, tt, ft):
        for eb in range(2):
            es = slice(eb * F, (eb + 1) * F)
            nc.tensor.matmul(
                po[:, es], ATN[ft][:, tt * P:(tt + 1) * P], wo[ft][:, es],
                start=(ft == 0), stop=(ft == FT - 1))

    def o_store(po, tt):
        ob = ob_p.tile([P, E], BF16, tag="ob", name="ob")
        if tt % 2 == 0:
            nc.scalar.activation(ob[:], po[:], AF.Copy)
            nc.gpsimd.dma_start(out.ap()[tt * P:(tt + 1) * P, :], ob[:])
        else:
            nc.vector.tensor_copy(ob[:], po[:])
            nc.sync.dma_start(out.ap()[tt * P:(tt + 1) * P, :], ob[:])

    po0 = psum.tile([P, S], F32, tag="e", name="pso", bufs=2)
    po1 = psum.tile([P, S], F32, tag="pv", name="pso", bufs=2)
    for ft in range(FT - 1):
        o_mm(po0, 0, ft)
        o_mm(po1, 1, ft)
    o_mm(po0, 0, FT - 1)
    o_store(po0, 0)
    o_mm(po1, 1, FT - 1)
    o_store(po1, 1)
    for tt in range(2, NKT):
        tg = "e" if tt % 2 == 0 else "pv"
        po = psum.tile([P, S], F32, tag=tg, name="pso", bufs=2)
        for ft in range(FT):
            o_mm(po, tt, ft)
        o_store(po, tt)


def build_nc(repeats=1, hw_loop=0):
    nc = bacc.Bacc()
    t = _declare(nc)
    with tile.TileContext(nc) as tc:
        with ExitStack() as ctx:
            a = _alloc(nc, tc, ctx)
            if hw_loop:
                with tc.For_i(0, hw_loop, 1):
                    _emit_body(nc, tc, t, a)
            else:
                for _ in range(repeats):
                    _emit_body(nc, tc, t, a)
    nc.finalize()
    return nc


_NC = None


def _get_nc():
    global _NC
    if _NC is None:
        _NC = build_nc()
    return _NC


def _prep_in_maps(value, key_in, query, mask, Wq, bq, Wk, bk, Wv, bv, Wo, bo):
    f = np.float32
    value = np.asarray(value, f)
    key_in = np.asarray(key_in, f)
    query = np.asarray(query, f)
    mask = np.asarray(mask)
    Wq = np.asarray(Wq, f); bq = np.asarray(bq, f)
    Wk = np.asarray(Wk, f); bk = np.asarray(bk, f)
    Wv = np.asarray(Wv, f); bv = np.asarray(bv, f)
    Wo = np.asarray(Wo, f)

    s = f(1.0 / np.sqrt(E))
    xqT = [query[b].T.astype(BF16_NP) for b in range(B)]
    xkT = [key_in[b].T.astype(BF16_NP) for b in range(B)]
    xvT = [value[b].T.astype(BF16_NP) for b in range(B)]
    wqTs = (Wq.T * s).astype(BF16_NP)
    wkT = Wk.T.astype(BF16_NP)
    wvT = Wv.T.astype(BF16_NP)
    woT = Wo.T.astype(BF16_NP)
    # per (batch, head-half): [x.T | w-slice] concatenated along columns
    xqw = [[np.ascontiguousarray(np.concatenate(
        [xqT[b], wqTs[:, hh * F:(hh + 1) * F]], axis=1))
        for hh in range(2)] for b in range(B)]
    xkw = [[np.ascontiguousarray(np.concatenate(
        [xkT[b], wkT[:, hh * F:(hh + 1) * F]], axis=1))
        for hh in range(2)] for b in range(B)]
    xvw = [[np.ascontiguousarray(np.concatenate(
        [xvT[b], wvT[:, hh * F:(hh + 1) * F]], axis=1))
        for hh in range(2)] for b in range(B)]
    wo_h = [np.ascontiguousarray(woT[hh * F:(hh + 1) * F, :])
            for hh in range(2)]
    bq_h = [(bq[hh * F:(hh + 1) * F] * s).reshape(FT, P).T for hh in range(2)]
    bk_h = [bk[hh * F:(hh + 1) * F].reshape(FT, P).T for hh in range(2)]
    mb = [np.where(mask[b, 0, 0, :] == 0, f(-50.0), f(0.0)).astype(f)
          .reshape(NKT, P).T for b in range(B)]
    bqkm = [[np.ascontiguousarray(np.concatenate(
        [bq_h[hh], bk_h[hh], mb[b]], axis=1)) for hh in range(2)]
        for b in range(B)]
    ones = np.ones((P, HC), BF16_NP)

    in_maps = []
    for c in range(N_CORES):
        b, hh = c // 2, c % 2
        in_maps.append({
            "xqw": xqw[b][hh], "xkw": xkw[b][hh], "xvw": xvw[b][hh],
            "woT": wo_h[hh],
            "bqkm": bqkm[b][hh],
            "ones": ones,
        })
    return in_maps


def _assemble(results, Wo, bv, bo):
    bo_eff = (np.asarray(bo, np.float64)
              + np.asarray(Wo, np.float64) @ np.asarray(bv, np.float64))
    out = np.empty((B, S, E), np.float32)
    for b in range(B):
        out[b] = (results[2 * b]["out"].astype(np.float64)
                  + results[2 * b + 1]["out"].astype(np.float64)
                  + bo_eff).astype(np.float32)
    return out


def kernel(value, key_in, query, mask, Wq, bq, Wk, bk, Wv, bv, Wo, bo):
    nc = _get_nc()
    in_maps = _prep_in_maps(value, key_in, query, mask,
                            Wq, bq, Wk, bk, Wv, bv, Wo, bo)
    r = run_bass_kernel_spmd(nc, in_maps, list(range(N_CORES)))
    return _assemble(r.results, Wo, bv, bo)


# revision 5
# speedup vs baseline: 1.1540x; 1.1540x over previous
"""Multi-head attention Trainium2 kernel v2 (nn_MultiHeadAttention_7035156430929).

B=4, S=1024, E=1024, H=16, D=64. Sharding: 8 cores = 4 batches x 2
head-halves (tensor parallel per the hint). Each core computes 8 heads
(a 512-feature slice of Q/K/V and the matching 512 rows of Wo) for all
1024 queries/keys of its batch, producing a PARTIAL output [1024, 1024].
Host sums the two partials per batch and adds bo + Wo@bv (the softmax
rows sum to 1, so the V bias folds into a constant output bias).

vs the v1 baseline: no duplicated K/V projection work (262144 PE rows
per core instead of 327680), all matmul operands bf16 (half the HBM
traffic, full PE rate), exp in [128,1024] double-PSUM-bank tiles (64
ACT instructions instead of 128), the V ones-column filled by a DVE
copy instead of a 2048-descriptor strided DMA, merged [x.T | w] input
DMAs (36 instead of 64 per iteration), pools/tiles allocated outside
the hardware loop so iterations pipeline (no per-iteration drain), and
a software-pipelined emission order: Q/K projection chains staggered to
hide their evictions, V-projection and previous-head PV matmuls woven
into the exp-paced score loops, and the last head's PV overlapped with
the first output-projection chains.

Engine assignment: PE matmuls; ACT = exp + half the O evictions;
DVE = Q/K/V evictions + reciprocal + normalize mult + half the O
evictions; Pool = denominator partition-broadcast + const/output DMAs;
SP = streamed input loads. PSUM: tag "e" 2x[128,1024] + tag "pv"
2x[65..128,1024] = all 8 banks.
"""
import sys

sys.path.insert(0, "/opt/trn_rl_repo")

from contextlib import ExitStack

import numpy as np
import ml_dtypes

import concourse.bacc as bacc
import concourse.tile as tile
from concourse import mybir
from concourse.bass_utils import run_bass_kernel_spmd

B, S, E, H, D = 4, 1024, 1024, 16, 64
P = 128
F = 512            # features (heads*D) per core
FT = F // P        # 4 feature tiles
ET = E // P        # 8 embedding tiles
NKT = S // P       # 8 key-token tiles
HC = 8             # heads per core
DP1 = D + 1        # V columns per head incl. ones column
N_CORES = 8
F32 = mybir.dt.float32
BF16 = mybir.dt.bfloat16
AF = mybir.ActivationFunctionType
BF16_NP = ml_dtypes.bfloat16


def _declare(nc):
    dp = nc.declare_dram_parameter
    t = {}
    # activations concatenated with this core's weight slice: [x.T | w]
    t["xqw"] = dp("xqw", [E, S + F], BF16, isOutput=False)
    t["xkw"] = dp("xkw", [E, S + F], BF16, isOutput=False)
    t["xvw"] = dp("xvw", [E, S + F], BF16, isOutput=False)
    t["woT"] = dp("woT", [F, E], BF16, isOutput=False)   # (feat, e_out)
    t["bqkm"] = dp("bqkm", [P, 2 * FT + NKT], F32, isOutput=False)
    t["ones"] = dp("ones", [P, HC], BF16, isOutput=False)
    t["out"] = dp("out", [S, E], BF16, isOutput=True)    # PARTIAL output
    return t


class _Tiles:
    pass


def _alloc(nc, tc, ctx, t):
    """Pools + persistent tiles, created once OUTSIDE the hardware loop so
    iterations pipeline (no per-iteration pool-boundary drain)."""
    a = _Tiles()
    const = ctx.enter_context(tc.tile_pool(name="const", bufs=1))
    a.bqkm_t = const.tile([P, 2 * FT + NKT], F32, tag="bqkm", name="bqkm")
    a.on_t = const.tile([P, HC], BF16, tag="on", name="on")

    xq_p = ctx.enter_context(tc.tile_pool(name="xq", bufs=ET))
    xk_p = ctx.enter_context(tc.tile_pool(name="xk", bufs=ET))
    xv_p = ctx.enter_context(tc.tile_pool(name="xv", bufs=ET))
    wo_p = ctx.enter_context(tc.tile_pool(name="wo", bufs=2))
    qt_p = ctx.enter_context(tc.tile_pool(name="qt", bufs=FT))
    kt_p = ctx.enter_context(tc.tile_pool(name="kt", bufs=FT))
    va_p = ctx.enter_context(tc.tile_pool(name="va", bufs=NKT))
    atn_p = ctx.enter_context(tc.tile_pool(name="atn", bufs=FT))
    a.pt_p = ctx.enter_context(tc.tile_pool(name="pt", bufs=12))
    a.rec_p = ctx.enter_context(tc.tile_pool(name="rec", bufs=2))
    a.bc_p = ctx.enter_context(tc.tile_pool(name="bc", bufs=2))
    a.ob_p = ctx.enter_context(tc.tile_pool(name="ob", bufs=2))
    a.psum = ctx.enter_context(tc.tile_pool(name="psum", bufs=2, space="PSUM"))

    a.xqw = [xq_p.tile([P, S + F], BF16, tag="xq", name="xqw")
             for _ in range(ET)]
    a.xkw = [xk_p.tile([P, S + F], BF16, tag="xk", name="xkw")
             for _ in range(ET)]
    a.xvw = [xv_p.tile([P, S + F], BF16, tag="xv", name="xvw")
             for _ in range(ET)]
    a.xq = [tl[:, 0:S] for tl in a.xqw]
    a.xk = [tl[:, 0:S] for tl in a.xkw]
    a.xv = [tl[:, 0:S] for tl in a.xvw]
    a.wq = [tl[:, S:S + F] for tl in a.xqw]
    a.wk = [tl[:, S:S + F] for tl in a.xkw]
    a.wv = [tl[:, S:S + F] for tl in a.xvw]
    a.wo2 = [wo_p.tile([P, 2 * E], BF16, tag="wo", name="wo2")
             for _ in range(2)]
    a.wo = [a.wo2[ft // 2][:, (ft % 2) * E:(ft % 2 + 1) * E]
            for ft in range(FT)]
    a.QT = [qt_p.tile([P, S], BF16, tag="qt", name="qt") for _ in range(FT)]
    a.KT = [kt_p.tile([P, S], BF16, tag="kt", name="kt") for _ in range(FT)]
    a.VA = [va_p.tile([P, HC * DP1], BF16, tag="va", name="va")
            for _ in range(NKT)]
    a.ATN = [atn_p.tile([P, S], BF16, tag="atn", name="atn")
             for _ in range(FT)]

    # one-time prologue: constant loads and the VA ones-columns (never
    # overwritten by the per-iteration V evictions, which only touch the
    # numerator columns)
    nc.gpsimd.dma_start(a.bqkm_t[:], t["bqkm"].ap()[:])
    nc.gpsimd.dma_start(a.on_t[:], t["ones"].ap()[:])
    for tt in range(NKT):
        va3 = a.VA[tt][:].rearrange("p (h c) -> p h c", c=DP1)
        nc.vector.tensor_copy(
            va3[:, :, D:DP1], a.on_t[:].rearrange("p (h c) -> p h c", c=1))
    return a


def _emit_body(nc, tc, t, a):
    xqw_d, xkw_d, xvw_d = t["xqw"], t["xkw"], t["xvw"]
    woT = t["woT"]
    ones, out = t["ones"], t["out"]
    bq_t = a.bqkm_t[:, 0:FT]
    bk_t = a.bqkm_t[:, FT:2 * FT]
    mb_t = a.bqkm_t[:, 2 * FT:2 * FT + NKT]
    on_t = a.on_t
    pt_p, rec_p, bc_p, ob_p, psum = a.pt_p, a.rec_p, a.bc_p, a.ob_p, a.psum
    xq, xk, xv = a.xq, a.xk, a.xv
    wq, wk, wv, wo = a.wq, a.wk, a.wv, a.wo
    QT, KT, VA, ATN = a.QT, a.KT, a.VA, a.ATN

    # Input loads in consumption order on the SP (sync) queue.
    for et in range(ET):
        nc.sync.dma_start(a.xqw[et][:], xqw_d.ap()[et * P:(et + 1) * P, :])
    for et in range(ET):
        nc.sync.dma_start(a.xkw[et][:], xkw_d.ap()[et * P:(et + 1) * P, :])
    for et in range(ET):
        nc.sync.dma_start(a.xvw[et][:], xvw_d.ap()[et * P:(et + 1) * P, :])
    for wi in range(2):
        nc.sync.dma_start(
            a.wo2[wi][:].rearrange("p (t e) -> p t e", t=2),
            woT.ap()[wi * 2 * P:(wi + 1) * 2 * P, :]
            .rearrange("(t p) e -> p t e", p=P))

    # ---- Q/K projections, interleaved so each pair's PSUM eviction (DVE)
    # hides under the other projection's matmuls: Qp0, Kp0, Qp1, Kp1.
    def proj_pair(w, x, dst, bias, fp):
        # ft0's chain runs 4 et-steps ahead of ft1's, so ft0's eviction is
        # fully hidden under ft1's tail and the next pair never stalls.
        ps = [psum.tile([P, S], F32, tag="e", name="pse") for _ in range(2)]

        def step(f2, et):
            ft = fp * 2 + f2
            for qb in range(2):
                qs = slice(qb * F, (qb + 1) * F)
                nc.tensor.matmul(
                    ps[f2][:, qs], w[et][:, ft * P:(ft + 1) * P],
                    x[et][:, qs],
                    start=(et == 0), stop=(et == ET - 1))

        def evict(f2):
            ft = fp * 2 + f2
            nc.vector.tensor_scalar_add(dst[ft][:], ps[f2][:],
                                        bias[:, ft:ft + 1])

        for et in range(4):
            step(0, et)
        for i in range(4):
            step(0, 4 + i)
            step(1, i)
        evict(0)
        for et in range(4, ET):
            step(1, et)
        evict(1)

    proj_pair(wq, xq, QT, bq_t, 0)
    proj_pair(wk, xk, KT, bk_t, 0)
    proj_pair(wq, xq, QT, bq_t, 1)
    proj_pair(wk, xk, KT, bk_t, 1)

    # ---- V projection work generator: woven into the (ACT-paced) score
    # loops of attention streams 0-1 to fill PE idle slots. Each unit is a
    # couple of matmuls or an eviction; VA[tt] complete before PV(x=0).
    def v_units():
        for tt in range(NKT):
            ps = psum.tile([P, F], F32, tag="e", name="psv")
            for et in range(0, ET, 2):
                def mm2(ps=ps, tt=tt, et=et):
                    for e2 in (et, et + 1):
                        nc.tensor.matmul(
                            ps[:], xv[e2][:, tt * P:(tt + 1) * P], wv[e2][:],
                            start=(e2 == 0), stop=(e2 == ET - 1))
                yield mm2
            def evict(ps=ps, tt=tt):
                va3 = VA[tt][:].rearrange("p (h c) -> p h c", c=DP1)
                ps3 = ps[:].rearrange("p (h c) -> p h c", c=D)
                nc.vector.tensor_copy(va3[:, :, 0:D], ps3)
            yield evict

    vgen = v_units()

    # ---- Attention: stream x = head h (j = x//2 f-tile, hp partition).
    # Stream x's score loop (ACT-paced by exp) has the PV matmuls of
    # stream x-1 and V-projection quanta woven into its PE idle slots.
    all_pts = {}

    def normalize(x, pv):
        j, hh2 = divmod(x, 2)
        hp = hh2 * D
        # q-halves: finer subtile deps let O-proj columns start earlier
        for qb in range(2):
            qs = slice(qb * F, (qb + 1) * F)
            rec = rec_p.tile([1, F], F32, tag="rec", name="rec")
            bc = bc_p.tile([D, F], F32, tag="bc", name="bc")
            nc.vector.reciprocal(rec[:], pv[D:DP1, qs])
            nc.gpsimd.partition_broadcast(bc[:], rec[:])
            nc.vector.tensor_mul(ATN[j][hp:hp + D, qs], pv[0:D, qs], bc[:])

    def emit_stream(x):
        j, hh2 = divmod(x, 2)
        hp = hh2 * D
        prev = all_pts.pop(x - 1, None)
        if prev is not None:
            pv_prev = psum.tile([DP1, S], F32, tag="pv", name="pspv", bufs=2)
        pts = []
        for kt in range(NKT):
            pe = psum.tile([P, S], F32, tag="e", name="pse")
            for qb in range(2):
                nc.tensor.matmul(
                    pe[:, qb * F:(qb + 1) * F],
                    KT[j][hp:hp + D, kt * P:(kt + 1) * P],
                    QT[j][hp:hp + D, qb * F:(qb + 1) * F],
                    start=True, stop=True, tile_position=(hp, 0))
            pt = pt_p.tile([P, S], BF16, tag="pt", name="pt")
            nc.scalar.activation(pt[:], pe[:], AF.Exp,
                                 bias=mb_t[:, kt:kt + 1])
            pts.append(pt)
            if prev is not None:
                for qb in range(2):
                    qs = slice(qb * F, (qb + 1) * F)
                    nc.tensor.matmul(
                        pv_prev[:, qs],
                        VA[kt][:, (x - 1) * DP1:x * DP1], prev[kt][:, qs],
                        start=(kt == 0), stop=(kt == NKT - 1))
            for _ in range(3):
                u = next(vgen, None)
                if u is not None:
                    u()
        if prev is not None:
            normalize(x - 1, pv_prev)
        all_pts[x] = pts

    for x in range(HC):
        emit_stream(x)
        if x == 1:
            # all V-projection units must precede the first PV reads
            assert next(vgen, None) is None, "V weave not exhausted"
    # final stream's PV runs unwoven (O-proj matmuls queue right behind)
    x = HC - 1
    pts = all_pts.pop(x)
    pv = psum.tile([DP1, S], F32, tag="pv", name="pspv", bufs=2)
    for kt in range(NKT):
        for qb in range(2):
            qs = slice(qb * F, (qb + 1) * F)
            nc.tensor.matmul(
                pv[:, qs], VA[kt][:, x * DP1:(x + 1) * DP1], pts[kt][:, qs],
                start=(kt == 0), stop=(kt == NKT - 1))
    normalize(x, pv)

    # ---- Output projection: out[tt] = sum_ft ATN[ft][:,tt].T @ wo[ft].
    # Alternate e/pv PSUM tags for a deeper eviction rotation. The first
    # two chains' ft0-2 steps are emitted before any ft3 step, filling PE
    # while the final stream's normalize chain completes ATN[3].
    def o_mm(po, tt, ft):
        for eb in range(2):
            es = slice(eb * F, (eb + 1) * F)
            nc.tensor.matmul(
                po[:, es], ATN[ft][:, tt * P:(tt + 1) * P], wo[ft][:, es],
                start=(ft == 0), stop=(ft == FT - 1))

    def o_store(po, tt):
        ob = ob_p.tile([P, E], BF16, tag="ob", name="ob")
        if tt % 2 == 0:
            nc.scalar.activation(ob[:], po[:], AF.Copy)
        else:
            nc.vector.tensor_copy(ob[:], po[:])
        nc.gpsimd.dma_start(out.ap()[tt * P:(tt + 1) * P, :], ob[:])

    po0 = psum.tile([P, S], F32, tag="e", name="pso", bufs=2)
    po1 = psum.tile([P, S], F32, tag="pv", name="pso", bufs=2)
    for ft in range(FT - 1):
        o_mm(po0, 0, ft)
        o_mm(po1, 1, ft)
    o_mm(po0, 0, FT - 1)
    o_store(po0, 0)
    o_mm(po1, 1, FT - 1)
    o_store(po1, 1)
    for tt in range(2, NKT):
        tg = "e" if tt % 2 == 0 else "pv"
        po = psum.tile([P, S], F32, tag=tg, name="pso", bufs=2)
        for ft in range(FT):
            o_mm(po, tt, ft)
        o_store(po, tt)


def build_nc(repeats=1, hw_loop=0):
    nc = bacc.Bacc()
    t = _declare(nc)
    with tile.TileContext(nc) as tc:
        with ExitStack() as ctx:
            a = _alloc(nc, tc, ctx, t)
            if hw_loop:
                with tc.For_i(0, hw_loop, 1):
                    _emit_body(nc, tc, t, a)
            else:
                for _ in range(repeats):
                    _emit_body(nc, tc, t, a)
    nc.finalize()
    return nc


_NC = None


def _get_nc():
    global _NC
    if _NC is None:
        _NC = build_nc()
    return _NC


def _prep_in_maps(value, key_in, query, mask, Wq, bq, Wk, bk, Wv, bv, Wo, bo):
    f = np.float32
    value = np.asarray(value, f)
    key_in = np.asarray(key_in, f)
    query = np.asarray(query, f)
    mask = np.asarray(mask)
    Wq = np.asarray(Wq, f); bq = np.asarray(bq, f)
    Wk = np.asarray(Wk, f); bk = np.asarray(bk, f)
    Wv = np.asarray(Wv, f); bv = np.asarray(bv, f)
    Wo = np.asarray(Wo, f)

    s = f(1.0 / np.sqrt(E))
    xqT = [query[b].T.astype(BF16_NP) for b in range(B)]
    xkT = [key_in[b].T.astype(BF16_NP) for b in range(B)]
    xvT = [value[b].T.astype(BF16_NP) for b in range(B)]
    wqTs = (Wq.T * s).astype(BF16_NP)
    wkT = Wk.T.astype(BF16_NP)
    wvT = Wv.T.astype(BF16_NP)
    woT = Wo.T.astype(BF16_NP)
    # per (batch, head-half): [x.T | w-slice] concatenated along columns
    xqw = [[np.ascontiguousarray(np.concatenate(
        [xqT[b], wqTs[:, hh * F:(hh + 1) * F]], axis=1))
        for hh in range(2)] for b in range(B)]
    xkw = [[np.ascontiguousarray(np.concatenate(
        [xkT[b], wkT[:, hh * F:(hh + 1) * F]], axis=1))
        for hh in range(2)] for b in range(B)]
    xvw = [[np.ascontiguousarray(np.concatenate(
        [xvT[b], wvT[:, hh * F:(hh + 1) * F]], axis=1))
        for hh in range(2)] for b in range(B)]
    wo_h = [np.ascontiguousarray(woT[hh * F:(hh + 1) * F, :])
            for hh in range(2)]
    bq_h = [(bq[hh * F:(hh + 1) * F] * s).reshape(FT, P).T for hh in range(2)]
    bk_h = [bk[hh * F:(hh + 1) * F].reshape(FT, P).T for hh in range(2)]
    mb = [np.where(mask[b, 0, 0, :] == 0, f(-50.0), f(0.0)).astype(f)
          .reshape(NKT, P).T for b in range(B)]
    bqkm = [[np.ascontiguousarray(np.concatenate(
        [bq_h[hh], bk_h[hh], mb[b]], axis=1)) for hh in range(2)]
        for b in range(B)]
    ones = np.ones((P, HC), BF16_NP)

    in_maps = []
    for c in range(N_CORES):
        b, hh = c // 2, c % 2
        in_maps.append({
            "xqw": xqw[b][hh], "xkw": xkw[b][hh], "xvw": xvw[b][hh],
            "woT": wo_h[hh],
            "bqkm": bqkm[b][hh],
            "ones": ones,
        })
    return in_maps


def _assemble(results, Wo, bv, bo):
    bo_eff = (np.asarray(bo, np.float64)
              + np.asarray(Wo, np.float64) @ np.asarray(bv, np.float64))
    out = np.empty((B, S, E), np.float32)
    for b in range(B):
        out[b] = (results[2 * b]["out"].astype(np.float64)
                  + results[2 * b + 1]["out"].astype(np.float64)
                  + bo_eff).astype(np.float32)
    return out


def kernel(value, key_in, query, mask, Wq, bq, Wk, bk, Wv, bv, Wo, bo):
    nc = _get_nc()
    in_maps = _prep_in_maps(value, key_in, query, mask,
                            Wq, bq, Wk, bk, Wv, bv, Wo, bo)
    r = run_bass_kernel_spmd(nc, in_maps, list(range(N_CORES)))
    return _assemble(r.results, Wo, bv, bo)


# revision 6
# speedup vs baseline: 1.4442x; 1.2515x over previous
"""Multi-head attention Trainium2 kernel v2 (nn_MultiHeadAttention_7035156430929).

B=4, S=1024, E=1024, H=16, D=64. Sharding: 8 cores = 4 batches x 2
head-halves (tensor parallel per the hint). Each core computes 8 heads
(a 512-feature slice of Q/K/V and the matching 512 rows of Wo) for all
1024 queries/keys of its batch, producing a PARTIAL output [1024, 1024].
Host sums the two partials per batch and adds bo + Wo@bv (the softmax
rows sum to 1, so the V bias folds into a constant output bias).

vs the v1 baseline: no duplicated K/V projection work (262144 PE rows
per core instead of 327680), all matmul operands bf16 (half the HBM
traffic, full PE rate), exp in [128,1024] double-PSUM-bank tiles (64
ACT instructions instead of 128), the V ones-column filled by a DVE
copy instead of a 2048-descriptor strided DMA, merged [x.T | w] input
DMAs (36 instead of 64 per iteration), pools/tiles allocated outside
the hardware loop so iterations pipeline (no per-iteration drain), and
a software-pipelined emission order: Q/K projection chains staggered to
hide their evictions, V-projection and previous-head PV matmuls woven
into the exp-paced score loops, and the last head's PV overlapped with
the first output-projection chains.

Engine assignment: PE matmuls; ACT = exp + half the O evictions;
DVE = Q/K/V evictions + reciprocal + normalize mult + half the O
evictions; Pool = denominator partition-broadcast + const/output DMAs;
SP = streamed input loads. PSUM: tag "e" 2x[128,1024] + tag "pv"
2x[65..128,1024] = all 8 banks.
"""
import sys

sys.path.insert(0, "/opt/trn_rl_repo")

from contextlib import ExitStack

import numpy as np
import ml_dtypes

import concourse.bacc as bacc
import concourse.tile as tile
from concourse import mybir
from concourse.bass_utils import run_bass_kernel_spmd

B, S, E, H, D = 4, 1024, 1024, 16, 64
P = 128
F = 512            # features (heads*D) per core
FT = F // P        # 4 feature tiles
ET = E // P        # 8 embedding tiles
NKT = S // P       # 8 key-token tiles
HC = 8             # heads per core
DP1 = D + 1        # V columns per head incl. ones column
N_CORES = 8
F32 = mybir.dt.float32
BF16 = mybir.dt.bfloat16
AF = mybir.ActivationFunctionType
BF16_NP = ml_dtypes.bfloat16


def _declare(nc):
    dp = nc.declare_dram_parameter
    t = {}
    # activations concatenated with this core's weight slice: [x.T | w]
    t["xqw"] = dp("xqw", [E, S + F], BF16, isOutput=False)
    t["xkw"] = dp("xkw", [E, S + F], BF16, isOutput=False)
    t["xvw"] = dp("xvw", [E, S + F], BF16, isOutput=False)
    t["woT"] = dp("woT", [F, E], BF16, isOutput=False)   # (feat, e_out)
    t["bqkm"] = dp("bqkm", [P, 2 * FT + NKT], F32, isOutput=False)
    t["ones"] = dp("ones", [P, HC], BF16, isOutput=False)
    t["out"] = dp("out", [S, E], BF16, isOutput=True)    # PARTIAL output
    return t


class _Tiles:
    pass


def _alloc(nc, tc, ctx, t):
    """Pools + persistent tiles, created once OUTSIDE the hardware loop so
    iterations pipeline (no per-iteration pool-boundary drain)."""
    a = _Tiles()
    const = ctx.enter_context(tc.tile_pool(name="const", bufs=1))
    a.bqkm_t = const.tile([P, 2 * FT + NKT], F32, tag="bqkm", name="bqkm")
    a.on_t = const.tile([P, HC], BF16, tag="on", name="on")

    xq_p = ctx.enter_context(tc.tile_pool(name="xq", bufs=ET))
    xk_p = ctx.enter_context(tc.tile_pool(name="xk", bufs=ET))
    xv_p = ctx.enter_context(tc.tile_pool(name="xv", bufs=ET))
    wo_p = ctx.enter_context(tc.tile_pool(name="wo", bufs=2))
    qt_p = ctx.enter_context(tc.tile_pool(name="qt", bufs=FT))
    kt_p = ctx.enter_context(tc.tile_pool(name="kt", bufs=FT))
    va_p = ctx.enter_context(tc.tile_pool(name="va", bufs=NKT))
    atn_p = ctx.enter_context(tc.tile_pool(name="atn", bufs=FT))
    a.pt_p = ctx.enter_context(tc.tile_pool(name="pt", bufs=14))
    a.rec_p = ctx.enter_context(tc.tile_pool(name="rec", bufs=4))
    a.bc_p = ctx.enter_context(tc.tile_pool(name="bc", bufs=4))
    a.ob_p = ctx.enter_context(tc.tile_pool(name="ob", bufs=3))
    a.psum = ctx.enter_context(tc.tile_pool(name="psum", bufs=2, space="PSUM"))

    a.xqw = [xq_p.tile([P, S + F], BF16, tag="xq", name="xqw")
             for _ in range(ET)]
    a.xkw = [xk_p.tile([P, S + F], BF16, tag="xk", name="xkw")
             for _ in range(ET)]
    a.xvw = [xv_p.tile([P, S + F], BF16, tag="xv", name="xvw")
             for _ in range(ET)]
    a.xq = [tl[:, 0:S] for tl in a.xqw]
    a.xk = [tl[:, 0:S] for tl in a.xkw]
    a.xv = [tl[:, 0:S] for tl in a.xvw]
    a.wq = [tl[:, S:S + F] for tl in a.xqw]
    a.wk = [tl[:, S:S + F] for tl in a.xkw]
    a.wv = [tl[:, S:S + F] for tl in a.xvw]
    a.wo2 = [wo_p.tile([P, 2 * E], BF16, tag="wo", name="wo2")
             for _ in range(2)]
    a.wo = [a.wo2[ft // 2][:, (ft % 2) * E:(ft % 2 + 1) * E]
            for ft in range(FT)]
    a.QT = [qt_p.tile([P, S], BF16, tag="qt", name="qt") for _ in range(FT)]
    a.KT = [kt_p.tile([P, S], BF16, tag="kt", name="kt") for _ in range(FT)]
    a.VA = [va_p.tile([P, HC * DP1], BF16, tag="va", name="va")
            for _ in range(NKT)]
    a.ATN = [atn_p.tile([P, S], BF16, tag="atn", name="atn")
             for _ in range(FT)]

    # one-time prologue: constant loads and the VA ones-columns (never
    # overwritten by the per-iteration V evictions, which only touch the
    # numerator columns)
    nc.gpsimd.dma_start(a.bqkm_t[:], t["bqkm"].ap()[:])
    nc.gpsimd.dma_start(a.on_t[:], t["ones"].ap()[:])
    for tt in range(NKT):
        va3 = a.VA[tt][:].rearrange("p (h c) -> p h c", c=DP1)
        nc.vector.tensor_copy(
            va3[:, :, D:DP1], a.on_t[:].rearrange("p (h c) -> p h c", c=1))
    return a


def _emit_body(nc, tc, t, a):
    xqw_d, xkw_d, xvw_d = t["xqw"], t["xkw"], t["xvw"]
    woT = t["woT"]
    ones, out = t["ones"], t["out"]
    bq_t = a.bqkm_t[:, 0:FT]
    bk_t = a.bqkm_t[:, FT:2 * FT]
    mb_t = a.bqkm_t[:, 2 * FT:2 * FT + NKT]
    on_t = a.on_t
    pt_p, rec_p, bc_p, ob_p, psum = a.pt_p, a.rec_p, a.bc_p, a.ob_p, a.psum
    xq, xk, xv = a.xq, a.xk, a.xv
    wq, wk, wv, wo = a.wq, a.wk, a.wv, a.wo
    QT, KT, VA, ATN = a.QT, a.KT, a.VA, a.ATN

    # Input loads in consumption order on the SP (sync) queue.
    for et in range(ET):
        nc.sync.dma_start(a.xqw[et][:], xqw_d.ap()[et * P:(et + 1) * P, :])
    for et in range(ET):
        nc.sync.dma_start(a.xkw[et][:], xkw_d.ap()[et * P:(et + 1) * P, :])
    for et in range(ET):
        nc.sync.dma_start(a.xvw[et][:], xvw_d.ap()[et * P:(et + 1) * P, :])
    for wi in range(2):
        nc.sync.dma_start(
            a.wo2[wi][:].rearrange("p (t e) -> p t e", t=2),
            woT.ap()[wi * 2 * P:(wi + 1) * 2 * P, :]
            .rearrange("(t p) e -> p t e", p=P))

    # ---- Q/K projections, interleaved so each pair's PSUM eviction (DVE)
    # hides under the other projection's matmuls: Qp0, Kp0, Qp1, Kp1.
    def proj_pair(w, x, dst, bias, fp):
        # ft0's chain runs 4 et-steps ahead of ft1's, so ft0's eviction is
        # fully hidden under ft1's tail and the next pair never stalls.
        ps = [psum.tile([P, S], F32, tag="e", name="pse") for _ in range(2)]

        def step(f2, et):
            ft = fp * 2 + f2
            for qb in range(2):
                qs = slice(qb * F, (qb + 1) * F)
                nc.tensor.matmul(
                    ps[f2][:, qs], w[et][:, ft * P:(ft + 1) * P],
                    x[et][:, qs],
                    start=(et == 0), stop=(et == ET - 1))

        def evict(f2):
            ft = fp * 2 + f2
            nc.vector.tensor_scalar_add(dst[ft][:], ps[f2][:],
                                        bias[:, ft:ft + 1])

        for et in range(4):
            step(0, et)
        for i in range(4):
            step(0, 4 + i)
            step(1, i)
        evict(0)
        for et in range(4, ET):
            step(1, et)
        evict(1)

    proj_pair(wq, xq, QT, bq_t, 0)
    proj_pair(wk, xk, KT, bk_t, 0)
    proj_pair(wq, xq, QT, bq_t, 1)
    proj_pair(wk, xk, KT, bk_t, 1)

    # ---- V projection work generator: woven into the (ACT-paced) score
    # loops of attention streams 0-1 to fill PE idle slots. Each unit is a
    # couple of matmuls or an eviction; VA[tt] complete before PV(x=0).
    def v_units():
        for tt in range(NKT):
            ps = psum.tile([P, F], F32, tag="e", name="psv")
            for et in range(0, ET, 2):
                def mm2(ps=ps, tt=tt, et=et):
                    for e2 in (et, et + 1):
                        nc.tensor.matmul(
                            ps[:], xv[e2][:, tt * P:(tt + 1) * P], wv[e2][:],
                            start=(e2 == 0), stop=(e2 == ET - 1))
                yield mm2
            def evict(ps=ps, tt=tt):
                va3 = VA[tt][:].rearrange("p (h c) -> p h c", c=DP1)
                ps3 = ps[:].rearrange("p (h c) -> p h c", c=D)
                nc.vector.tensor_copy(va3[:, :, 0:D], ps3)
            yield evict

    vgen = v_units()

    # ---- Attention: stream x = head h (j = x//2 f-tile, hp partition).
    # Stream x's score loop (ACT-paced by exp) has the PV matmuls of
    # stream x-1 and V-projection quanta woven into its PE idle slots.
    all_pts = {}

    def normalize(x, pv):
        j, hh2 = divmod(x, 2)
        hp = hh2 * D
        # q-halves: finer subtile deps let O-proj columns start earlier
        for qb in range(2):
            qs = slice(qb * F, (qb + 1) * F)
            rec = rec_p.tile([1, F], F32, tag="rec", name="rec")
            bc = bc_p.tile([D, F], F32, tag="bc", name="bc")
            nc.vector.reciprocal(rec[:], pv[D:DP1, qs])
            nc.gpsimd.partition_broadcast(bc[:], rec[:])
            nc.vector.tensor_mul(ATN[j][hp:hp + D, qs], pv[0:D, qs], bc[:])

    def emit_stream(x):
        j, hh2 = divmod(x, 2)
        hp = hh2 * D
        prev = all_pts.pop(x - 1, None)
        if prev is not None:
            pv_prev = psum.tile([DP1, S], F32, tag="pv", name="pspv", bufs=2)
        pts = []
        for kt in range(NKT):
            pe = psum.tile([P, S], F32, tag="e", name="pse")
            for qb in range(2):
                nc.tensor.matmul(
                    pe[:, qb * F:(qb + 1) * F],
                    KT[j][hp:hp + D, kt * P:(kt + 1) * P],
                    QT[j][hp:hp + D, qb * F:(qb + 1) * F],
                    start=True, stop=True, tile_position=(hp, 0))
            pt = pt_p.tile([P, S], BF16, tag="pt", name="pt")
            nc.scalar.activation(pt[:], pe[:], AF.Exp,
                                 bias=mb_t[:, kt:kt + 1])
            pts.append(pt)
            if prev is not None:
                for qb in range(2):
                    qs = slice(qb * F, (qb + 1) * F)
                    nc.tensor.matmul(
                        pv_prev[:, qs],
                        VA[kt][:, (x - 1) * DP1:x * DP1], prev[kt][:, qs],
                        start=(kt == 0), stop=(kt == NKT - 1))
            for _ in range(3):
                u = next(vgen, None)
                if u is not None:
                    u()
        if prev is not None:
            normalize(x - 1, pv_prev)
        all_pts[x] = pts

    for x in range(HC):
        emit_stream(x)
        if x == 1:
            # all V-projection units must precede the first PV reads
            assert next(vgen, None) is None, "V weave not exhausted"
    # final stream's PV runs unwoven (O-proj matmuls queue right behind)
    x = HC - 1
    pts = all_pts.pop(x)
    pv = psum.tile([DP1, S], F32, tag="pv", name="pspv", bufs=2)
    for kt in range(NKT):
        for qb in range(2):
            qs = slice(qb * F, (qb + 1) * F)
            nc.tensor.matmul(
                pv[:, qs], VA[kt][:, x * DP1:(x + 1) * DP1], pts[kt][:, qs],
                start=(kt == 0), stop=(kt == NKT - 1))
    normalize(x, pv)

    # ---- Output projection: out[tt] = sum_ft ATN[ft][:,tt].T @ wo[ft].
    # Alternate e/pv PSUM tags for a deeper eviction rotation. The first
    # two chains' ft0-2 steps are emitted before any ft3 step, filling PE
    # while the final stream's normalize chain completes ATN[3].
    def o_mm(po, tt, ft):
        for eb in range(2):
            es = slice(eb * F, (eb + 1) * F)
            nc.tensor.matmul(
                po[:, es], ATN[ft][:, tt * P:(tt + 1) * P], wo[ft][:, es],
                start=(ft == 0), stop=(ft == FT - 1))

    def o_store(po, tt):
        ob = ob_p.tile([P, E], BF16, tag="ob", name="ob")
        if tt % 2 == 0:
            nc.scalar.activation(ob[:], po[:], AF.Copy)
        else:
            nc.vector.tensor_copy(ob[:], po[:])
        nc.gpsimd.dma_start(out.ap()[tt * P:(tt + 1) * P, :], ob[:])

    po0 = psum.tile([P, S], F32, tag="e", name="pso", bufs=2)
    po1 = psum.tile([P, S], F32, tag="pv", name="pso", bufs=2)
    for ft in range(FT - 1):
        o_mm(po0, 0, ft)
        o_mm(po1, 1, ft)
    o_mm(po0, 0, FT - 1)
    o_store(po0, 0)
    o_mm(po1, 1, FT - 1)
    o_store(po1, 1)
    for tt in range(2, NKT):
        tg = "e" if tt % 2 == 0 else "pv"
        po = psum.tile([P, S], F32, tag=tg, name="pso", bufs=2)
        for ft in range(FT):
            o_mm(po, tt, ft)
        o_store(po, tt)


def build_nc(repeats=1, hw_loop=0):
    nc = bacc.Bacc()
    t = _declare(nc)
    with tile.TileContext(nc) as tc:
        with ExitStack() as ctx:
            a = _alloc(nc, tc, ctx, t)
            if hw_loop:
                with tc.For_i(0, hw_loop, 1):
                    _emit_body(nc, tc, t, a)
            else:
                for _ in range(repeats):
                    _emit_body(nc, tc, t, a)
    nc.finalize()
    return nc


_NC = None


def _get_nc():
    global _NC
    if _NC is None:
        _NC = build_nc()
    return _NC


def _prep_in_maps(value, key_in, query, mask, Wq, bq, Wk, bk, Wv, bv, Wo, bo):
    f = np.float32
    value = np.asarray(value, f)
    key_in = np.asarray(key_in, f)
    query = np.asarray(query, f)
    mask = np.asarray(mask)
    Wq = np.asarray(Wq, f); bq = np.asarray(bq, f)
    Wk = np.asarray(Wk, f); bk = np.asarray(bk, f)
    Wv = np.asarray(Wv, f); bv = np.asarray(bv, f)
    Wo = np.asarray(Wo, f)

    s = f(1.0 / np.sqrt(E))
    xqT = [query[b].T.astype(BF16_NP) for b in range(B)]
    xkT = [key_in[b].T.astype(BF16_NP) for b in range(B)]
    xvT = [value[b].T.astype(BF16_NP) for b in range(B)]
    wqTs = (Wq.T * s).astype(BF16_NP)
    wkT = Wk.T.astype(BF16_NP)
    wvT = Wv.T.astype(BF16_NP)
    woT = Wo.T.astype(BF16_NP)
    # per (batch, head-half): [x.T | w-slice] concatenated along columns
    xqw = [[np.ascontiguousarray(np.concatenate(
        [xqT[b], wqTs[:, hh * F:(hh + 1) * F]], axis=1))
        for hh in range(2)] for b in range(B)]
    xkw = [[np.ascontiguousarray(np.concatenate(
        [xkT[b], wkT[:, hh * F:(hh + 1) * F]], axis=1))
        for hh in range(2)] for b in range(B)]
    xvw = [[np.ascontiguousarray(np.concatenate(
        [xvT[b], wvT[:, hh * F:(hh + 1) * F]], axis=1))
        for hh in range(2)] for b in range(B)]
    wo_h = [np.ascontiguousarray(woT[hh * F:(hh + 1) * F, :])
            for hh in range(2)]
    bq_h = [(bq[hh * F:(hh + 1) * F] * s).reshape(FT, P).T for hh in range(2)]
    bk_h = [bk[hh * F:(hh + 1) * F].reshape(FT, P).T for hh in range(2)]
    mb = [np.where(mask[b, 0, 0, :] == 0, f(-50.0), f(0.0)).astype(f)
          .reshape(NKT, P).T for b in range(B)]
    bqkm = [[np.ascontiguousarray(np.concatenate(
        [bq_h[hh], bk_h[hh], mb[b]], axis=1)) for hh in range(2)]
        for b in range(B)]
    ones = np.ones((P, HC), BF16_NP)

    in_maps = []
    for c in range(N_CORES):
        b, hh = c // 2, c % 2
        in_maps.append({
            "xqw": xqw[b][hh], "xkw": xkw[b][hh], "xvw": xvw[b][hh],
            "woT": wo_h[hh],
            "bqkm": bqkm[b][hh],
            "ones": ones,
        })
    return in_maps


def _assemble(results, Wo, bv, bo):
    bo_eff = (np.asarray(bo, np.float64)
              + np.asarray(Wo, np.float64) @ np.asarray(bv, np.float64))
    out = np.empty((B, S, E), np.float32)
    for b in range(B):
        out[b] = (results[2 * b]["out"].astype(np.float64)
                  + results[2 * b + 1]["out"].astype(np.float64)
                  + bo_eff).astype(np.float32)
    return out


def kernel(value, key_in, query, mask, Wq, bq, Wk, bk, Wv, bv, Wo, bo):
    nc = _get_nc()
    in_maps = _prep_in_maps(value, key_in, query, mask,
                            Wq, bq, Wk, bk, Wv, bv, Wo, bo)
    r = run_bass_kernel_spmd(nc, in_maps, list(range(N_CORES)))
    return _assemble(r.results, Wo, bv, bo)


# revision 7
# speedup vs baseline: 1.4620x; 1.0124x over previous
"""Multi-head attention Trainium2 kernel v2 (nn_MultiHeadAttention_7035156430929).

B=4, S=1024, E=1024, H=16, D=64. Sharding: 8 cores = 4 batches x 2
head-halves (tensor parallel per the hint). Each core computes 8 heads
(a 512-feature slice of Q/K/V and the matching 512 rows of Wo) for all
1024 queries/keys of its batch, producing a PARTIAL output [1024, 1024].
Host sums the two partials per batch and adds bo + Wo@bv (the softmax
rows sum to 1, so the V bias folds into a constant output bias).

vs the v1 baseline: no duplicated K/V projection work (262144 PE rows
per core instead of 327680), all matmul operands bf16 (half the HBM
traffic, full PE rate), exp in [128,1024] double-PSUM-bank tiles (64
ACT instructions instead of 128), the V ones-column filled by a DVE
copy instead of a 2048-descriptor strided DMA, merged [x.T | w] input
DMAs (36 instead of 64 per iteration), pools/tiles allocated outside
the hardware loop so iterations pipeline (no per-iteration drain), and
a software-pipelined emission order: Q/K projection chains staggered to
hide their evictions, V-projection and previous-head PV matmuls woven
into the exp-paced score loops, and the last head's PV overlapped with
the first output-projection chains.

Engine assignment: PE matmuls; ACT = exp + half the O evictions;
DVE = Q/K/V evictions + reciprocal + normalize mult + half the O
evictions; Pool = denominator partition-broadcast + const/output DMAs;
SP = streamed input loads. PSUM: tag "e" 2x[128,1024] + tag "pv"
2x[65..128,1024] = all 8 banks.
"""
import sys

sys.path.insert(0, "/opt/trn_rl_repo")

from contextlib import ExitStack

import numpy as np
import ml_dtypes

import concourse.bacc as bacc
import concourse.tile as tile
from concourse import mybir
from concourse.bass_utils import run_bass_kernel_spmd

B, S, E, H, D = 4, 1024, 1024, 16, 64
P = 128
F = 512            # features (heads*D) per core
FT = F // P        # 4 feature tiles
ET = E // P        # 8 embedding tiles
NKT = S // P       # 8 key-token tiles
HC = 8             # heads per core
DP1 = D + 1        # V columns per head incl. ones column
N_CORES = 8
F32 = mybir.dt.float32
BF16 = mybir.dt.bfloat16
AF = mybir.ActivationFunctionType
BF16_NP = ml_dtypes.bfloat16


def _declare(nc, kp):
    nkte = kp // P
    dp = nc.declare_dram_parameter
    t = {}
    # activations concatenated with this core's weight slice: [x.T | w];
    # keys/values are mask-compacted to kp columns (masked keys carry
    # exactly zero softmax weight, so they are dropped host-side)
    t["xqw"] = dp("xqw", [E, S + F], BF16, isOutput=False)
    t["xkw"] = dp("xkw", [E, kp + F], BF16, isOutput=False)
    t["xvw"] = dp("xvw", [E, kp + F], BF16, isOutput=False)
    t["woT"] = dp("woT", [F, E], BF16, isOutput=False)   # (feat, e_out)
    t["bqkm"] = dp("bqkm", [P, 2 * FT + nkte], F32, isOutput=False)
    t["ones"] = dp("ones", [P, HC], BF16, isOutput=False)
    t["out"] = dp("out", [S, E], BF16, isOutput=True)    # PARTIAL output
    return t


class _Tiles:
    pass


def _alloc(nc, tc, ctx, t, kp):
    """Pools + persistent tiles, created once OUTSIDE the hardware loop so
    iterations pipeline (no per-iteration pool-boundary drain)."""
    a = _Tiles()
    nkte = kp // P
    const = ctx.enter_context(tc.tile_pool(name="const", bufs=1))
    a.bqkm_t = const.tile([P, 2 * FT + nkte], F32, tag="bqkm", name="bqkm")
    a.on_t = const.tile([P, HC], BF16, tag="on", name="on")

    xq_p = ctx.enter_context(tc.tile_pool(name="xq", bufs=ET))
    xk_p = ctx.enter_context(tc.tile_pool(name="xk", bufs=ET))
    xv_p = ctx.enter_context(tc.tile_pool(name="xv", bufs=ET))
    wo_p = ctx.enter_context(tc.tile_pool(name="wo", bufs=2))
    qt_p = ctx.enter_context(tc.tile_pool(name="qt", bufs=FT))
    kt_p = ctx.enter_context(tc.tile_pool(name="kt", bufs=FT))
    va_p = ctx.enter_context(tc.tile_pool(name="va", bufs=NKT))
    atn_p = ctx.enter_context(tc.tile_pool(name="atn", bufs=FT))
    a.pt_p = ctx.enter_context(tc.tile_pool(name="pt", bufs=14))
    a.rec_p = ctx.enter_context(tc.tile_pool(name="rec", bufs=4))
    a.bc_p = ctx.enter_context(tc.tile_pool(name="bc", bufs=4))
    a.ob_p = ctx.enter_context(tc.tile_pool(name="ob", bufs=3))
    a.psum = ctx.enter_context(tc.tile_pool(name="psum", bufs=2, space="PSUM"))

    a.xqw = [xq_p.tile([P, S + F], BF16, tag="xq", name="xqw")
             for _ in range(ET)]
    a.xkw = [xk_p.tile([P, kp + F], BF16, tag="xk", name="xkw")
             for _ in range(ET)]
    a.xvw = [xv_p.tile([P, kp + F], BF16, tag="xv", name="xvw")
             for _ in range(ET)]
    a.xq = [tl[:, 0:S] for tl in a.xqw]
    a.xk = [tl[:, 0:kp] for tl in a.xkw]
    a.xv = [tl[:, 0:kp] for tl in a.xvw]
    a.wq = [tl[:, S:S + F] for tl in a.xqw]
    a.wk = [tl[:, kp:kp + F] for tl in a.xkw]
    a.wv = [tl[:, kp:kp + F] for tl in a.xvw]
    a.wo2 = [wo_p.tile([P, 2 * E], BF16, tag="wo", name="wo2")
             for _ in range(2)]
    a.wo = [a.wo2[ft // 2][:, (ft % 2) * E:(ft % 2 + 1) * E]
            for ft in range(FT)]
    a.QT = [qt_p.tile([P, S], BF16, tag="qt", name="qt") for _ in range(FT)]
    a.KT = [kt_p.tile([P, kp], BF16, tag="kt", name="kt") for _ in range(FT)]
    a.VA = [va_p.tile([P, HC * DP1], BF16, tag="va", name="va")
            for _ in range(nkte)]
    a.ATN = [atn_p.tile([P, S], BF16, tag="atn", name="atn")
             for _ in range(FT)]

    # one-time prologue: constant loads and the VA ones-columns (never
    # overwritten by the per-iteration V evictions, which only touch the
    # numerator columns)
    nc.gpsimd.dma_start(a.bqkm_t[:], t["bqkm"].ap()[:])
    nc.gpsimd.dma_start(a.on_t[:], t["ones"].ap()[:])
    for tt in range(nkte):
        va3 = a.VA[tt][:].rearrange("p (h c) -> p h c", c=DP1)
        nc.vector.tensor_copy(
            va3[:, :, D:DP1], a.on_t[:].rearrange("p (h c) -> p h c", c=1))
    return a


def _emit_body(nc, tc, t, a, kp):
    nkte = kp // P
    xqw_d, xkw_d, xvw_d = t["xqw"], t["xkw"], t["xvw"]
    woT = t["woT"]
    ones, out = t["ones"], t["out"]
    bq_t = a.bqkm_t[:, 0:FT]
    bk_t = a.bqkm_t[:, FT:2 * FT]
    mb_t = a.bqkm_t[:, 2 * FT:2 * FT + nkte]
    on_t = a.on_t
    pt_p, rec_p, bc_p, ob_p, psum = a.pt_p, a.rec_p, a.bc_p, a.ob_p, a.psum
    xq, xk, xv = a.xq, a.xk, a.xv
    wq, wk, wv, wo = a.wq, a.wk, a.wv, a.wo
    QT, KT, VA, ATN = a.QT, a.KT, a.VA, a.ATN

    # Input loads in consumption order on the SP (sync) queue.
    for et in range(ET):
        nc.sync.dma_start(a.xqw[et][:], xqw_d.ap()[et * P:(et + 1) * P, :])
    for et in range(ET):
        nc.sync.dma_start(a.xkw[et][:], xkw_d.ap()[et * P:(et + 1) * P, :])
    for et in range(ET):
        nc.sync.dma_start(a.xvw[et][:], xvw_d.ap()[et * P:(et + 1) * P, :])
    for wi in range(2):
        nc.sync.dma_start(
            a.wo2[wi][:].rearrange("p (t e) -> p t e", t=2),
            woT.ap()[wi * 2 * P:(wi + 1) * 2 * P, :]
            .rearrange("(t p) e -> p t e", p=P))

    # ---- Q/K projections, interleaved so each pair's PSUM eviction (DVE)
    # hides under the other projection's matmuls: Qp0, Kp0, Qp1, Kp1.
    def proj_pair(w, x, dst, bias, fp, width):
        # ft0's chain runs 4 et-steps ahead of ft1's, so ft0's eviction is
        # fully hidden under ft1's tail and the next pair never stalls.
        ps = [psum.tile([P, S], F32, tag="e", name="pse") for _ in range(2)]
        chunks = [slice(c, min(c + F, width)) for c in range(0, width, F)]

        def step(f2, et):
            ft = fp * 2 + f2
            for qs in chunks:
                nc.tensor.matmul(
                    ps[f2][:, qs], w[et][:, ft * P:(ft + 1) * P],
                    x[et][:, qs],
                    start=(et == 0), stop=(et == ET - 1))

        def evict(f2):
            ft = fp * 2 + f2
            nc.vector.tensor_scalar_add(dst[ft][:, 0:width],
                                        ps[f2][:, 0:width],
                                        bias[:, ft:ft + 1])

        for et in range(4):
            step(0, et)
        for i in range(4):
            step(0, 4 + i)
            step(1, i)
        evict(0)
        for et in range(4, ET):
            step(1, et)
        evict(1)

    proj_pair(wq, xq, QT, bq_t, 0, S)
    proj_pair(wk, xk, KT, bk_t, 0, kp)
    proj_pair(wq, xq, QT, bq_t, 1, S)
    proj_pair(wk, xk, KT, bk_t, 1, kp)

    # ---- V projection work generator: woven into the (ACT-paced) score
    # loops of attention streams 0-1 to fill PE idle slots. Each unit is a
    # couple of matmuls or an eviction; VA[tt] complete before PV(x=0).
    def v_units():
        for tt in range(nkte):
            ps = psum.tile([P, F], F32, tag="e", name="psv")
            for et in range(0, ET, 2):
                def mm2(ps=ps, tt=tt, et=et):
                    for e2 in (et, et + 1):
                        nc.tensor.matmul(
                            ps[:], xv[e2][:, tt * P:(tt + 1) * P], wv[e2][:],
                            start=(e2 == 0), stop=(e2 == ET - 1))
                yield mm2
            def evict(ps=ps, tt=tt):
                va3 = VA[tt][:].rearrange("p (h c) -> p h c", c=DP1)
                ps3 = ps[:].rearrange("p (h c) -> p h c", c=D)
                nc.vector.tensor_copy(va3[:, :, 0:D], ps3)
            yield evict

    vgen = v_units()

    # ---- Attention: stream x = head h (j = x//2 f-tile, hp partition).
    # Stream x's score loop (ACT-paced by exp) has the PV matmuls of
    # stream x-1 and V-projection quanta woven into its PE idle slots.
    all_pts = {}

    def normalize(x, pv):
        j, hh2 = divmod(x, 2)
        hp = hh2 * D
        # q-halves: finer subtile deps let O-proj columns start earlier
        for qb in range(2):
            qs = slice(qb * F, (qb + 1) * F)
            rec = rec_p.tile([1, F], F32, tag="rec", name="rec")
            bc = bc_p.tile([D, F], F32, tag="bc", name="bc")
            nc.vector.reciprocal(rec[:], pv[D:DP1, qs])
            nc.gpsimd.partition_broadcast(bc[:], rec[:])
            nc.vector.tensor_mul(ATN[j][hp:hp + D, qs], pv[0:D, qs], bc[:])

    def emit_stream(x):
        j, hh2 = divmod(x, 2)
        hp = hh2 * D
        prev = all_pts.pop(x - 1, None)
        if prev is not None:
            pv_prev = psum.tile([DP1, S], F32, tag="pv", name="pspv", bufs=2)
        pts = []
        for kt in range(nkte):
            pe = psum.tile([P, S], F32, tag="e", name="pse")
            for qb in range(2):
                nc.tensor.matmul(
                    pe[:, qb * F:(qb + 1) * F],
                    KT[j][hp:hp + D, kt * P:(kt + 1) * P],
                    QT[j][hp:hp + D, qb * F:(qb + 1) * F],
                    start=True, stop=True, tile_position=(hp, 0))
            pt = pt_p.tile([P, S], BF16, tag="pt", name="pt")
            nc.scalar.activation(pt[:], pe[:], AF.Exp,
                                 bias=mb_t[:, kt:kt + 1])
            pts.append(pt)
            if prev is not None:
                for qb in range(2):
                    qs = slice(qb * F, (qb + 1) * F)
                    nc.tensor.matmul(
                        pv_prev[:, qs],
                        VA[kt][:, (x - 1) * DP1:x * DP1], prev[kt][:, qs],
                        start=(kt == 0), stop=(kt == nkte - 1))
            for _ in range(3):
                u = next(vgen, None)
                if u is not None:
                    u()
        if prev is not None:
            normalize(x - 1, pv_prev)
        all_pts[x] = pts

    for x in range(HC):
        emit_stream(x)
        if x == 1:
            # all V-projection units must precede the first PV reads
            assert next(vgen, None) is None, "V weave not exhausted"
    # final stream's PV runs unwoven (O-proj matmuls queue right behind)
    x = HC - 1
    pts = all_pts.pop(x)
    pv = psum.tile([DP1, S], F32, tag="pv", name="pspv", bufs=2)
    for kt in range(nkte):
        for qb in range(2):
            qs = slice(qb * F, (qb + 1) * F)
            nc.tensor.matmul(
                pv[:, qs], VA[kt][:, x * DP1:(x + 1) * DP1], pts[kt][:, qs],
                start=(kt == 0), stop=(kt == nkte - 1))
    normalize(x, pv)

    # ---- Output projection: out[tt] = sum_ft ATN[ft][:,tt].T @ wo[ft].
    # Alternate e/pv PSUM tags for a deeper eviction rotation. The first
    # two chains' ft0-2 steps are emitted before any ft3 step, filling PE
    # while the final stream's normalize chain completes ATN[3].
    def o_mm(po, tt, ft):
        for eb in range(2):
            es = slice(eb * F, (eb + 1) * F)
            nc.tensor.matmul(
                po[:, es], ATN[ft][:, tt * P:(tt + 1) * P], wo[ft][:, es],
                start=(ft == 0), stop=(ft == FT - 1))

    def o_store(po, tt):
        ob = ob_p.tile([P, E], BF16, tag="ob", name="ob")
        if tt % 2 == 0:
            nc.scalar.activation(ob[:], po[:], AF.Copy)
        else:
            nc.vector.tensor_copy(ob[:], po[:])
        nc.gpsimd.dma_start(out.ap()[tt * P:(tt + 1) * P, :], ob[:])

    po0 = psum.tile([P, S], F32, tag="e", name="pso", bufs=2)
    po1 = psum.tile([P, S], F32, tag="pv", name="pso", bufs=2)
    for ft in range(FT - 1):
        o_mm(po0, 0, ft)
        o_mm(po1, 1, ft)
    o_mm(po0, 0, FT - 1)
    o_store(po0, 0)
    o_mm(po1, 1, FT - 1)
    o_store(po1, 1)
    for tt in range(2, NKT):
        tg = "e" if tt % 2 == 0 else "pv"
        po = psum.tile([P, S], F32, tag=tg, name="pso", bufs=2)
        for ft in range(FT):
            o_mm(po, tt, ft)
        o_store(po, tt)


_LAST_KP = S  # key length of the most recent _prep_in_maps (benchmarks)


def build_nc(repeats=1, hw_loop=0, kp=None):
    if kp is None:
        kp = _LAST_KP
    nc = bacc.Bacc()
    t = _declare(nc, kp)
    with tile.TileContext(nc) as tc:
        with ExitStack() as ctx:
            a = _alloc(nc, tc, ctx, t, kp)
            if hw_loop:
                with tc.For_i(0, hw_loop, 1):
                    _emit_body(nc, tc, t, a, kp)
            else:
                for _ in range(repeats):
                    _emit_body(nc, tc, t, a, kp)
    nc.finalize()
    return nc


_NC = {}


def _get_nc(kp):
    if kp not in _NC:
        _NC[kp] = build_nc(kp=kp)
    return _NC[kp]


def _prep_in_maps(value, key_in, query, mask, Wq, bq, Wk, bk, Wv, bv, Wo, bo):
    f = np.float32
    value = np.asarray(value, f)
    key_in = np.asarray(key_in, f)
    query = np.asarray(query, f)
    mask = np.asarray(mask)
    Wq = np.asarray(Wq, f); bq = np.asarray(bq, f)
    Wk = np.asarray(Wk, f); bk = np.asarray(bk, f)
    Wv = np.asarray(Wv, f); bv = np.asarray(bv, f)
    Wo = np.asarray(Wo, f)

    s = f(1.0 / np.sqrt(E))
    global _LAST_KP
    # mask compaction: masked keys carry exactly zero softmax weight, so
    # keep only unmasked key/value columns, padded to a multiple of 128
    # (pad columns get the -50 exp bias). One shared kp across all cores
    # (SPMD: every core runs the same program).
    idxs = [np.flatnonzero(mask[b, 0, 0, :] != 0) for b in range(B)]
    if min(len(ix) for ix in idxs) >= 3 * P:
        kp = max(P * ((len(ix) + P - 1) // P) for ix in idxs)
        kp = min(kp, S)
    else:  # degenerate masks: fall back to the uncompacted path
        kp = S
        idxs = [np.arange(S) for _ in range(B)]
    _LAST_KP = kp
    nkte = kp // P

    def pad_cols(arrT):
        # arrT: [E, ke] -> [E, kp] zero-padded
        ke = arrT.shape[1]
        if ke == kp:
            return arrT
        return np.concatenate(
            [arrT, np.zeros((arrT.shape[0], kp - ke), arrT.dtype)], axis=1)

    xqT = [query[b].T.astype(BF16_NP) for b in range(B)]
    xkT = [pad_cols(key_in[b].T[:, idxs[b]].astype(BF16_NP))
           for b in range(B)]
    xvT = [pad_cols(value[b].T[:, idxs[b]].astype(BF16_NP))
           for b in range(B)]
    wqTs = (Wq.T * s).astype(BF16_NP)
    wkT = Wk.T.astype(BF16_NP)
    wvT = Wv.T.astype(BF16_NP)
    woT = Wo.T.astype(BF16_NP)
    # per (batch, head-half): [x.T | w-slice] concatenated along columns
    xqw = [[np.ascontiguousarray(np.concatenate(
        [xqT[b], wqTs[:, hh * F:(hh + 1) * F]], axis=1))
        for hh in range(2)] for b in range(B)]
    xkw = [[np.ascontiguousarray(np.concatenate(
        [xkT[b], wkT[:, hh * F:(hh + 1) * F]], axis=1))
        for hh in range(2)] for b in range(B)]
    xvw = [[np.ascontiguousarray(np.concatenate(
        [xvT[b], wvT[:, hh * F:(hh + 1) * F]], axis=1))
        for hh in range(2)] for b in range(B)]
    wo_h = [np.ascontiguousarray(woT[hh * F:(hh + 1) * F, :])
            for hh in range(2)]
    bq_h = [(bq[hh * F:(hh + 1) * F] * s).reshape(FT, P).T for hh in range(2)]
    bk_h = [bk[hh * F:(hh + 1) * F].reshape(FT, P).T for hh in range(2)]
    def mk_mb(b):
        m = np.full(kp, f(-50.0), f)
        m[0:len(idxs[b])] = f(0.0)
        if kp == S and len(idxs[b]) == S:  # uncompacted fallback
            m = np.where(mask[b, 0, 0, :] == 0, f(-50.0), f(0.0)).astype(f)
        return m.reshape(nkte, P).T

    mb = [mk_mb(b) for b in range(B)]
    bqkm = [[np.ascontiguousarray(np.concatenate(
        [bq_h[hh], bk_h[hh], mb[b]], axis=1)) for hh in range(2)]
        for b in range(B)]
    ones = np.ones((P, HC), BF16_NP)

    in_maps = []
    for c in range(N_CORES):
        b, hh = c // 2, c % 2
        in_maps.append({
            "xqw": xqw[b][hh], "xkw": xkw[b][hh], "xvw": xvw[b][hh],
            "woT": wo_h[hh],
            "bqkm": bqkm[b][hh],
            "ones": ones,
        })
    return in_maps


def _assemble(results, Wo, bv, bo):
    bo_eff = (np.asarray(bo, np.float64)
              + np.asarray(Wo, np.float64) @ np.asarray(bv, np.float64))
    out = np.empty((B, S, E), np.float32)
    for b in range(B):
        out[b] = (results[2 * b]["out"].astype(np.float64)
                  + results[2 * b + 1]["out"].astype(np.float64)
                  + bo_eff).astype(np.float32)
    return out


def kernel(value, key_in, query, mask, Wq, bq, Wk, bk, Wv, bv, Wo, bo):
    in_maps = _prep_in_maps(value, key_in, query, mask,
                            Wq, bq, Wk, bk, Wv, bv, Wo, bo)
    nc = _get_nc(_LAST_KP)
    r = run_bass_kernel_spmd(nc, in_maps, list(range(N_CORES)))
    return _assemble(r.results, Wo, bv, bo)
